# revision 39
# baseline (speedup 1.0000x reference)
"""Trainium2 Bass kernel for nn_BaselineAttn (LoRA QKV + ALiBi causal attention).

Sharding: 8 cores SPMD, no collectives. Core c = (b, g): batch b = c // 4,
head group g = c % 4 handling heads [g, 4+g, 8+g, 12+g].

Host prep: LoRA folded into weights (W' = W + 2 A@B); x and weights
pre-transposed, sliced per core, and PRE-TILED so each DMA is one large
contiguous transfer (9 input DMAs total; dma_start issue cost is ~600ns
flat, so fewer/bigger transfers start compute earlier and keep the PE
clock ramped).

Device design (fp16 operands, fp32 PSUM), chunk-interleaved pipeline:
  for qc in 0..3:  emit [qkv projections for token chunk qc]
                        [output projection for chunk qc-1]
                        [attention for chunk qc]
  so scalar-engine exp work and DMA overlap PE matmuls throughout, and
  the softmax-normalization DMA bounce latency always hides under the
  next chunk's projection matmuls.

  - attention in the S^T (key-major) orientation:
      S^T tile = k^T-tile.T @ q^T-chunk
      P^T = exp(S^T/8 + bias_k), bias_k = -slope_h*k per-PARTITION: ALiBi +
        softmax shift fused into one ScalarE activation.
      causal: diagonal-band tiles multiplied by a 0/1 mask (GpSimd);
        dead tiles skipped; per-tile active q-range sliced.
      O^T += (v|ones).T @ P^T  (ones column = softmax denominator in row 64)
      normalize: fast-reciprocal -> DRAM-bounce partition broadcast (on the
        sync DMA queue whose later work is latency-tolerant) -> DVE mul.
        The kernel-terminal slot instead broadcasts via a PE matmul
        (ones[1,64].T @ recip16) + scalar copy: no DMA latency.
      out-partial = O^T_norm.T @ Wp'^T-slice, written as fp16 partials.
  - ALiBi here rewards early keys: exp bias is -slope_h*k; keys with
    slope_h*k > ~30 are dropped (< 1e-4 of row mass worst case). Head->slot
    assignment keeps per-slot k-tile caps uniform: SNKT = [1, 4, 15, 16].
  - PSUM: 8 banks exactly: ring A (qk-proj acc + S^T) x3, ring B (v-proj
    pair acc + out-proj acc) x2, ring C (O^T) x3.
"""

import math

import numpy as np

E = 1024
H = 16
DH = 64
T = 2048
BATCH = 2
LORA_S = 2.0
NKT = T // 128          # 16 key tiles of 128
SNKT = [1, 4, 15, 16]   # per-slot key-tile caps (ALiBi cutoff slope*k > 30)
NQC = 4                 # q chunks of 512

_NC_CACHE = {}


def _slopes():
    start = 2 ** (-2 ** (-(math.log2(H) - 3)))
    return np.array([start * start**i for i in range(H)], dtype=np.float64)


def _smin(tt):
    """Lowest slot that still needs key-tile tt."""
    for s in range(4):
        if tt < SNKT[s]:
            return s
    return 4


def _build_nc():
    """Build the single SPMD Bass program (shared by all 8 cores)."""
    if "nc" in _NC_CACHE:
        return _NC_CACHE["nc"]

    from concourse.bacc import Bacc
    import concourse.tile as tile
    from concourse import mybir

    f16 = mybir.dt.float16
    f32 = mybir.dt.float32
    EXP = mybir.ActivationFunctionType.Exp

    nc = Bacc()

    # pre-tiled inputs: [128, ...] with kt-blocks side by side per partition
    x4_d = nc.dram_tensor("x4", [128, 4 * 4096], f16, kind="ExternalInput")
    wq_d = nc.dram_tensor("wqT", [128, 8 * 256], f16, kind="ExternalInput")
    wk_d = nc.dram_tensor("wkT", [128, 8 * 256], f16, kind="ExternalInput")
    wv_d = nc.dram_tensor("wvT", [128, 8 * 256], f16, kind="ExternalInput")
    wp_d = nc.dram_tensor("wpT", [128, 2 * 1024], f16, kind="ExternalInput")
    bias_d = nc.dram_tensor("expbias", [128, 64], f32, kind="ExternalInput")
    mask_d = nc.dram_tensor("masks", [128, 4 * 512], f16, kind="ExternalInput")
    out_d = nc.dram_tensor("outp", [T, E], f16, kind="ExternalOutput")
    rbounce_d = nc.dram_tensor("rbounce", [16, 512], f32, kind="Internal")

    with tile.TileContext(nc) as tc:
        with (
            tc.tile_pool(name="persist", bufs=1) as pp,
            tc.tile_pool(name="ptpool", bufs=10) as ptp,
            tc.tile_pool(name="onorm", bufs=4) as onp,
            tc.tile_pool(name="rpool", bufs=2) as rp,
            tc.tile_pool(name="bcpool", bufs=3) as bcp,
            tc.tile_pool(name="outsb", bufs=3) as osp,
            tc.tile_pool(name="pa", bufs=3, space="PSUM") as pa,
            tc.tile_pool(name="pb", bufs=2, space="PSUM") as pb,
            tc.tile_pool(name="pc", bufs=3, space="PSUM") as pc,
        ):
            wq_sb = pp.tile([128, 8 * 256], f16, name="wq")
            wk_sb = pp.tile([128, 8 * 256], f16, name="wk")
            wv_sb = pp.tile([128, 8 * 256], f16, name="wv")
            wp_sb = pp.tile([128, 2 * 1024], f16, name="wp")
            bias_sb = pp.tile([128, 64], f32, name="bias")
            mask_sb = pp.tile([128, 4 * 512], f16, name="mask")
            ones1 = pp.tile([1, 64], f16, name="ones1")
            nc.vector.memset(ones1, 1.0)
            xsb = [pp.tile([128, 4096], f16, name=f"x{c}") for c in range(NQC)]

            # input DMA order tuned so each consumer's data lands just in
            # time.  Per-queue DMA bandwidth is ~140GB/s (engines shared);
            # only sync/scalar can issue fast DMAs (gpsimd's queue is a
            # single slow engine - small transfers only).  Consumer order:
            # q groups (wq + x chunk 0), k groups (wk), v groups (wv).
            nc.sync.dma_start(out=wq_sb[:, 0:512], in_=wq_d[:, 0:512])
            nc.scalar.dma_start(out=xsb[0][:, 0:512], in_=x4_d[:, 0:512])
            nc.gpsimd.dma_start(out=bias_sb, in_=bias_d[:, :])
            nc.sync.dma_start(out=wq_sb[:, 512:1024], in_=wq_d[:, 512:1024])
            nc.scalar.dma_start(out=xsb[0][:, 512:1024], in_=x4_d[:, 512:1024])
            nc.sync.dma_start(out=xsb[0][:, 1024:2048], in_=x4_d[:, 1024:2048])
            nc.scalar.dma_start(out=xsb[0][:, 2048:3072], in_=x4_d[:, 2048:3072])
            nc.sync.dma_start(out=wq_sb[:, 1024:2048], in_=wq_d[:, 1024:2048])
            nc.scalar.dma_start(out=xsb[0][:, 3072:4096], in_=x4_d[:, 3072:4096])
            nc.sync.dma_start(out=wk_sb, in_=wk_d[:, :])
            nc.sync.dma_start(out=wv_sb, in_=wv_d[:, :])
            nc.scalar.dma_start(out=mask_sb, in_=mask_d[:, :])
            nc.sync.dma_start(out=xsb[1], in_=x4_d[:, 4096:8192])
            nc.scalar.dma_start(out=wp_sb, in_=wp_d[:, :])
            nc.sync.dma_start(out=xsb[2], in_=x4_d[:, 8192:12288])
            nc.sync.dma_start(out=xsb[3], in_=x4_d[:, 12288:16384])

            # vext ones preset (gpsimd, after its DMA issue; no data deps)
            vext = []
            for tt in range(NKT):
                v_t = pp.tile([128, 4, 65], f16, name=f"vext{tt}")
                nc.gpsimd.memset(v_t, 1.0)  # ones cols preset; v overwrites rest
                vext.append(v_t)

            # q^T / k^T: per (p-tile, chunk) tiles [128, 512].
            # kT p-tile 0 (slots 0,1) only needs k < 512: chunk 0 only.
            qT = [[pp.tile([128, 512], f16, name=f"qT{p}_{c}") for c in range(NQC)]
                  for p in range(2)]
            kT = [[pp.tile([128, 512], f16, name=f"kT{p}_{c}")
                   if (p == 1 or c < 1) else None for c in range(NQC)]
                  for p in range(2)]

            on_tiles = [None] * NQC  # per-qc [pt] normalized O^T, f16

            def qk_group(dst, wofs, mt, ncu, nw):
                """One q/k projection group: 8 matmuls + DVE copy to SBUF."""
                acc = pa.tile([128, 512], f32, tag="acc", name=f"a{wofs}_{mt}_{ncu}")
                for kt in range(8):
                    w_base = wq_sb if wofs == 0 else wk_sb
                    w_sl = w_base[:, kt * 256 + mt * 128:kt * 256 + (mt + 1) * 128]
                    nc.tensor.matmul(
                        acc[:, 0:nw], w_sl,
                        xsb[ncu][:, kt * 512:kt * 512 + nw],
                        start=(kt == 0), stop=(kt == 7),
                    )
                nc.vector.tensor_copy(out=dst[:, 0:nw], in_=acc[:, 0:nw])

            def v_pair(tt0, ncu):
                """v projection for token tiles tt0, tt0+1 sharing one bank."""
                acc = pb.tile([128, 512], f32, tag="vp", name=f"v{tt0}")
                cols = []
                for i, tt in enumerate((tt0, tt0 + 1)):
                    s0 = _smin(tt)
                    nw = (4 - s0) * 64
                    cols.append((tt, s0, nw))
                    for kt in range(8):
                        nc.tensor.matmul(
                            acc[:, i * 256:i * 256 + nw],
                            xsb[ncu][:, kt * 512 + (tt % 4) * 128:
                                     kt * 512 + (tt % 4 + 1) * 128],
                            wv_sb[:, kt * 256 + s0 * 64:kt * 256 + 256],
                            start=(kt == 0), stop=(kt == 7),
                        )
                for i, (tt, s0, nw) in enumerate(cols):
                    nc.vector.tensor_copy(
                        out=vext[tt][:, s0:4, 0:64],
                        in_=acc[:, i * 256:i * 256 + nw]
                        .rearrange("p (s d) -> p s d", d=64))

            def emit_chunk(ncu):
                # q groups first: they only need wq + x (k needs wkv, which
                # lands on its own queue a bit later at startup)
                with nc.named_scope(f"qkv_c{ncu}"):
                    qk_group(qT[1][ncu], 0, 1, ncu, 512)
                    qk_group(qT[0][ncu], 0, 0, ncu, 512)
                    qk_group(kT[1][ncu], 256, 1, ncu, 512)
                    v_pair(4 * ncu, ncu)
                    v_pair(4 * ncu + 2, ncu)
                    if ncu == 0:
                        qk_group(kT[0][0], 256, 0, 0, 512)

            def emit_proj(qc, last=False):
                with nc.named_scope(f"proj_q{qc}"):
                    for tloc in range(4):
                        tt = qc * 4 + tloc
                        osb = osp.tile([128, 1024], f16, tag="osb", name=f"o{tt}")
                        for ech in range(2):
                            # last proj: ring A is idle by now and has 3 slots
                            # (vs 2) - lets one more start-half matmul run
                            # ahead while the terminal normalize completes
                            pool, tag = (pa, "acc") if last else (pb, "vp")
                            pacc = pool.tile([128, 512], f32, tag=tag,
                                             name=f"pa_{tt}_{ech}")
                            for pt_i in range(2):
                                nc.tensor.matmul(
                                    pacc,
                                    on_tiles[qc][pt_i][:, tloc * 128:(tloc + 1) * 128],
                                    wp_sb[:, pt_i * 1024 + ech * 512:
                                          pt_i * 1024 + (ech + 1) * 512],
                                    start=(pt_i == 0), stop=(pt_i == 1),
                                )
                            nc.vector.tensor_copy(
                                out=osb[:, ech * 512:(ech + 1) * 512], in_=pacc)
                            if last:  # drain the tail on two queues, per half
                                eng = nc.sync if ech == 0 else nc.scalar
                                eng.dma_start(
                                    out=out_d[tt * 128:(tt + 1) * 128,
                                              ech * 512:(ech + 1) * 512],
                                    in_=osb[:, ech * 512:(ech + 1) * 512])
                        if not last:
                            nc.sync.dma_start(
                                out=out_d[tt * 128:(tt + 1) * 128, :], in_=osb)

            nmask = 0

            def emit_attention(qc):
                nonlocal nmask
                on_tiles[qc] = [onp.tile([128, 512], f16, tag="on",
                                         name=f"on_{qc}_{p}") for p in range(2)]
                # last chunk: small pair first so its normalize bounce hides
                # under the big pair's attention; the final slot's normalize
                # uses a PE broadcast (no DMA latency) right before proj.
                pair_order = (0, 1) if qc == NQC - 1 else (1, 0)
                for pair in pair_order:
                    ot_save = [None, None]
                    for s in (2 * pair + 1, 2 * pair):
                        nkt = min(SNKT[s], 4 * qc + 4)
                        r0 = 64 * (s % 2)
                        ot = pc.tile([128, 512], f32, tag="ot", name=f"ot_{qc}_{s}")
                        # terminal pair only: masked diag tiles first, so the
                        # slot ends with clean tiles and the final normalize
                        # chain starts ~1us earlier (global reorder regresses)
                        if qc == NQC - 1 and pair == pair_order[1]:
                            kts = ([k for k in range(nkt) if k >= 4 * qc]
                                   + [k for k in range(nkt) if k < 4 * qc])
                        else:
                            kts = list(range(nkt))
                        with nc.named_scope(f"attn_q{qc}_s{s}"):
                            for ki, kt in enumerate(kts):
                                j0 = (kt - 4 * qc) * 128 if kt >= 4 * qc else 0
                                st = pa.tile([128, 512], f32, tag="acc",
                                             name=f"st_{qc}_{s}_{kt}")
                                nc.tensor.matmul(
                                    st[:, j0:512],
                                    kT[pair][kt // 4][r0:r0 + 64,
                                                      (kt % 4) * 128:(kt % 4 + 1) * 128],
                                    qT[pair][qc][r0:r0 + 64, j0:512],
                                    start=True, stop=True,
                                )
                                p_t = ptp.tile([128, 512], f16, tag="pt",
                                               name=f"pt_{qc}_{s}_{kt}")
                                nc.scalar.activation(
                                    out=p_t[:, j0:512], in_=st[:, j0:512],
                                    func=EXP,
                                    bias=bias_sb[:, s * 16 + kt:s * 16 + kt + 1],
                                    scale=0.125,
                                )
                                if kt >= 4 * qc:
                                    m = kt - 4 * qc
                                    nmask += 1
                                    # split diag-burst masks across gpsimd+DVE.
                                    # Only pair-1 (emitted before the pair's
                                    # DVE norm-muls) may use DVE at qc0: pair-0
                                    # DVE masks would queue behind norm-muls
                                    # stalled on the bounce DMA.
                                    meng = (nc.vector
                                            if (m % 2 == 1 and (qc >= 1 or s >= 2))
                                            else nc.gpsimd)
                                    meng.tensor_mul(
                                        out=p_t[:, j0:512],
                                        in0=p_t[:, j0:512],
                                        in1=mask_sb[:, m * 512 + j0:(m + 1) * 512],
                                    )
                                nc.tensor.matmul(
                                    ot[0:65, j0:512],
                                    vext[kt][:, s, :],
                                    p_t[:, j0:512],
                                    start=(ki == 0), stop=(ki == len(kts) - 1),
                                )
                            ot_save[s % 2] = ot
                    # denominators -> fast reciprocal -> DRAM-bounce partition
                    # broadcast (on the sync queue: its other work, output
                    # tiles, is not latency-critical) -> DVE normalize
                    with nc.named_scope(f"norm_q{qc}_p{pair}"):
                        bcs = bcp.tile([128, 512], f32, tag="bcs",
                                       name=f"b_{qc}_{pair}")
                        for s in (2 * pair + 1, 2 * pair):
                            r0 = 64 * (s % 2)
                            sums = rp.tile([1, 512], f32, tag="sum",
                                           name=f"s_{qc}_{s}")
                            nc.vector.tensor_copy(out=sums,
                                                  in_=ot_save[s % 2][64:65, :])
                            recip = rp.tile([1, 512], f32, tag="rcp",
                                            name=f"r_{qc}_{s}")
                            nc.vector.reciprocal_approx_fast(out=recip, in_=sums)
                            if qc == NQC - 1 and pair == pair_order[1] and s % 2 == 0:
                                # terminal slot: PE matmul broadcast, ~3us
                                # faster than the DMA bounce round trip
                                recip16 = rp.tile([1, 512], f16, tag="r16",
                                                  name=f"r16_{qc}_{s}")
                                nc.vector.tensor_copy(out=recip16, in_=recip)
                                bc_ps = pa.tile([128, 512], f32, tag="acc",
                                                name=f"bps_{qc}_{s}")
                                nc.tensor.matmul(bc_ps[r0:r0 + 64, :],
                                                 ones1[0:1, 0:64], recip16,
                                                 start=True, stop=True)
                                nc.scalar.copy(out=bcs[r0:r0 + 64, :],
                                               in_=bc_ps[r0:r0 + 64, :])
                            else:
                                row = 4 * qc + s
                                nc.sync.dma_start(out=rbounce_d[row:row + 1, :],
                                                  in_=recip)
                                nc.sync.dma_start(
                                    out=bcs[r0:r0 + 64, :],
                                    in_=rbounce_d[row:row + 1, :]
                                    .to_broadcast([64, 512]))
                            nc.vector.tensor_mul(
                                out=on_tiles[qc][pair][r0:r0 + 64, :],
                                in0=ot_save[s % 2][0:64, :],
                                in1=bcs[r0:r0 + 64, :],
                            )

            # last iteration: attention before proj(q2) so the final
            # normalize chains hide under projection matmuls
            for ncu in range(NQC):
                emit_chunk(ncu)
                if 1 <= ncu < NQC - 1:
                    emit_proj(ncu - 1)
                emit_attention(ncu)
            emit_proj(NQC - 2)
            emit_proj(NQC - 1, last=True)

    nc.finalize()
    _NC_CACHE["nc"] = nc
    return nc


def _tile_cols(arr, nblk, blk):
    """[nblk*128, blk] -> [128, nblk*blk] with kt-blocks side by side."""
    n = arr.shape[0] // 128
    assert n == nblk and arr.shape[1] == blk
    return np.ascontiguousarray(
        arr.reshape(nblk, 128, blk).transpose(1, 0, 2).reshape(128, nblk * blk))


def _prep_core_inputs(x, Wq, Aq, Bq, Wk, Ak, Bk, Wv, Av, Bv, Wp):
    """Host-side prep: LoRA fold, transposes, per-core pre-tiled slices."""
    slopes = _slopes()
    wq_m = Wq.astype(np.float64) + LORA_S * (Aq.astype(np.float64) @ Bq.astype(np.float64))
    wk_m = Wk.astype(np.float64) + LORA_S * (Ak.astype(np.float64) @ Bk.astype(np.float64))
    wv_m = Wv.astype(np.float64) + LORA_S * (Av.astype(np.float64) @ Bv.astype(np.float64))

    # mask_m[p, j] = 1 if (m*128 + p) <= j else 0   (j in 0..511)
    p_i = np.arange(128)[:, None]
    j_i = np.arange(512)[None, :]
    masks = np.ascontiguousarray(np.concatenate(
        [((m * 128 + p_i) <= j_i).astype(np.float16) for m in range(4)], axis=1))

    in_maps = []
    for c in range(8):
        b, g = divmod(c, 4)
        heads = [g, 4 + g, 8 + g, 12 + g]
        rows = np.concatenate([np.arange(h * DH, (h + 1) * DH) for h in heads])
        xT = x[b].T.astype(np.float16)          # [E, T]
        x4 = np.concatenate(
            [_tile_cols(np.ascontiguousarray(xT[:, c0 * 512:(c0 + 1) * 512]), 8, 512)
             for c0 in range(NQC)], axis=1)     # [128, 4*4096]
        wqT = _tile_cols(wq_m[rows, :].T.astype(np.float16), 8, 256)
        wkT = _tile_cols(wk_m[rows, :].T.astype(np.float16), 8, 256)
        wvT = _tile_cols(wv_m[rows, :].T.astype(np.float16), 8, 256)
        wpT = _tile_cols(Wp[:, rows].T.astype(np.float16), 2, 1024)
        bias = np.zeros((128, 64), dtype=np.float32)
        for s, h in enumerate(heads):
            for kt in range(16):
                bias[:, s * 16 + kt] = -slopes[h] * (kt * 128 + np.arange(128))
        in_maps.append({
            "x4": np.ascontiguousarray(x4), "wqT": wqT, "wkT": wkT, "wvT": wvT,
            "wpT": wpT, "expbias": bias, "masks": masks,
        })
    return in_maps


def _run(in_maps, trace=False, **kw):
    from concourse.bass_utils import run_bass_kernel_spmd
    nc = _build_nc()
    return run_bass_kernel_spmd(nc, in_maps, core_ids=list(range(8)), trace=trace, **kw)


def kernel(x, Wq, Aq, Bq, Wk, Ak, Bk, Wv, Av, Bv, Wp):
    in_maps = _prep_core_inputs(x, Wq, Aq, Bq, Wk, Ak, Bk, Wv, Av, Bv, Wp)
    res = _run(in_maps)
    out = np.zeros((BATCH, T, E), dtype=np.float32)
    for c in range(8):
        out[c // 4] += res.results[c]["outp"].astype(np.float32)
    return out


# revision 40
# speedup vs baseline: 1.0099x; 1.0099x over previous
"""Trainium2 Bass kernel for nn_BaselineAttn (LoRA QKV + ALiBi causal attention).

Sharding: 8 cores SPMD, no collectives. Core c = (b, g): batch b = c // 4,
head group g = c % 4 handling heads [g, 4+g, 8+g, 12+g].

Host prep: LoRA folded into weights (W' = W + 2 A@B); x and weights
pre-transposed, sliced per core, and PRE-TILED so each DMA is one large
contiguous transfer (9 input DMAs total; dma_start issue cost is ~600ns
flat, so fewer/bigger transfers start compute earlier and keep the PE
clock ramped).

Device design (fp16 operands, fp32 PSUM), chunk-interleaved pipeline:
  for qc in 0..3:  emit [qkv projections for token chunk qc]
                        [output projection for chunk qc-1]
                        [attention for chunk qc]
  so scalar-engine exp work and DMA overlap PE matmuls throughout, and
  the softmax-normalization DMA bounce latency always hides under the
  next chunk's projection matmuls.

  - attention in the S^T (key-major) orientation:
      S^T tile = k^T-tile.T @ q^T-chunk
      P^T = exp(S^T/8 + bias_k), bias_k = -slope_h*k per-PARTITION: ALiBi +
        softmax shift fused into one ScalarE activation.
      causal: diagonal-band tiles multiplied by a 0/1 mask (GpSimd);
        dead tiles skipped; per-tile active q-range sliced.
      O^T += (v|ones).T @ P^T  (ones column = softmax denominator in row 64)
      normalize: fast-reciprocal -> DRAM-bounce partition broadcast (on the
        sync DMA queue whose later work is latency-tolerant) -> DVE mul.
        The kernel-terminal slot instead broadcasts via a PE matmul
        (ones[1,64].T @ recip16) + scalar copy: no DMA latency.
      out-partial = O^T_norm.T @ Wp'^T-slice, written as fp16 partials.
  - ALiBi here rewards early keys: exp bias is -slope_h*k; keys with
    slope_h*k > ~30 are dropped (< 1e-4 of row mass worst case). Head->slot
    assignment keeps per-slot k-tile caps uniform: SNKT = [1, 4, 15, 16].
  - PSUM: 8 banks exactly: ring A (qk-proj acc + S^T) x3, ring B (v-proj
    pair acc + out-proj acc) x2, ring C (O^T) x3.
"""

import math

import numpy as np

E = 1024
H = 16
DH = 64
T = 2048
BATCH = 2
LORA_S = 2.0
NKT = T // 128          # 16 key tiles of 128
SNKT = [1, 4, 15, 16]   # per-slot key-tile caps (ALiBi cutoff slope*k > 30)
NQC = 4                 # q chunks of 512

_NC_CACHE = {}


def _slopes():
    start = 2 ** (-2 ** (-(math.log2(H) - 3)))
    return np.array([start * start**i for i in range(H)], dtype=np.float64)


def _smin(tt):
    """Lowest slot that still needs key-tile tt."""
    for s in range(4):
        if tt < SNKT[s]:
            return s
    return 4


def _build_nc():
    """Build the single SPMD Bass program (shared by all 8 cores)."""
    if "nc" in _NC_CACHE:
        return _NC_CACHE["nc"]

    from concourse.bacc import Bacc
    import concourse.tile as tile
    from concourse import mybir

    f16 = mybir.dt.float16
    f32 = mybir.dt.float32
    EXP = mybir.ActivationFunctionType.Exp

    nc = Bacc()

    # pre-tiled inputs: [128, ...] with kt-blocks side by side per partition
    x4_d = nc.dram_tensor("x4", [128, 4 * 4096], f16, kind="ExternalInput")
    wq_d = nc.dram_tensor("wqT", [128, 8 * 256], f16, kind="ExternalInput")
    wk_d = nc.dram_tensor("wkT", [128, 8 * 256], f16, kind="ExternalInput")
    wv_d = nc.dram_tensor("wvT", [128, 8 * 256], f16, kind="ExternalInput")
    wp_d = nc.dram_tensor("wpT", [128, 2 * 1024], f16, kind="ExternalInput")
    bias_d = nc.dram_tensor("expbias", [128, 64], f32, kind="ExternalInput")
    mask_d = nc.dram_tensor("masks", [128, 4 * 512], f16, kind="ExternalInput")
    out_d = nc.dram_tensor("outp", [T, E], f16, kind="ExternalOutput")
    rbounce_d = nc.dram_tensor("rbounce", [16, 512], f32, kind="Internal")

    with tile.TileContext(nc) as tc:
        with (
            tc.tile_pool(name="persist", bufs=1) as pp,
            tc.tile_pool(name="ptpool", bufs=12) as ptp,
            tc.tile_pool(name="onorm", bufs=4) as onp,
            tc.tile_pool(name="rpool", bufs=2) as rp,
            tc.tile_pool(name="bcpool", bufs=4) as bcp,
            tc.tile_pool(name="outsb", bufs=3) as osp,
            tc.tile_pool(name="pa", bufs=3, space="PSUM") as pa,
            tc.tile_pool(name="pb", bufs=2, space="PSUM") as pb,
            tc.tile_pool(name="pc", bufs=3, space="PSUM") as pc,
        ):
            wq_sb = pp.tile([128, 8 * 256], f16, name="wq")
            wk_sb = pp.tile([128, 8 * 256], f16, name="wk")
            wv_sb = pp.tile([128, 8 * 256], f16, name="wv")
            wp_sb = pp.tile([128, 2 * 1024], f16, name="wp")
            bias_sb = pp.tile([128, 64], f32, name="bias")
            mask_sb = pp.tile([128, 4 * 512], f16, name="mask")
            ones1 = pp.tile([1, 64], f16, name="ones1")
            nc.vector.memset(ones1, 1.0)
            xsb = [pp.tile([128, 4096], f16, name=f"x{c}") for c in range(NQC)]

            # input DMA order tuned so each consumer's data lands just in
            # time.  Per-queue DMA bandwidth is ~140GB/s (engines shared);
            # only sync/scalar can issue fast DMAs (gpsimd's queue is a
            # single slow engine - small transfers only).  Consumer order:
            # q groups (wq + x chunk 0), k groups (wk), v groups (wv).
            nc.sync.dma_start(out=wq_sb[:, 0:512], in_=wq_d[:, 0:512])
            nc.scalar.dma_start(out=xsb[0][:, 0:512], in_=x4_d[:, 0:512])
            nc.gpsimd.dma_start(out=bias_sb, in_=bias_d[:, :])
            nc.sync.dma_start(out=wq_sb[:, 512:1024], in_=wq_d[:, 512:1024])
            nc.scalar.dma_start(out=xsb[0][:, 512:1024], in_=x4_d[:, 512:1024])
            nc.sync.dma_start(out=xsb[0][:, 1024:2048], in_=x4_d[:, 1024:2048])
            nc.scalar.dma_start(out=xsb[0][:, 2048:3072], in_=x4_d[:, 2048:3072])
            nc.sync.dma_start(out=wq_sb[:, 1024:2048], in_=wq_d[:, 1024:2048])
            nc.scalar.dma_start(out=xsb[0][:, 3072:4096], in_=x4_d[:, 3072:4096])
            nc.sync.dma_start(out=wk_sb, in_=wk_d[:, :])
            nc.sync.dma_start(out=wv_sb, in_=wv_d[:, :])
            nc.scalar.dma_start(out=mask_sb, in_=mask_d[:, :])
            nc.sync.dma_start(out=xsb[1], in_=x4_d[:, 4096:8192])
            nc.scalar.dma_start(out=wp_sb, in_=wp_d[:, :])
            nc.sync.dma_start(out=xsb[2], in_=x4_d[:, 8192:12288])
            nc.sync.dma_start(out=xsb[3], in_=x4_d[:, 12288:16384])

            # vext ones preset (gpsimd, after its DMA issue; no data deps)
            vext = []
            for tt in range(NKT):
                v_t = pp.tile([128, 4, 65], f16, name=f"vext{tt}")
                nc.gpsimd.memset(v_t, 1.0)  # ones cols preset; v overwrites rest
                vext.append(v_t)

            # q^T / k^T: per (p-tile, chunk) tiles [128, 512].
            # kT p-tile 0 (slots 0,1) only needs k < 512: chunk 0 only.
            qT = [[pp.tile([128, 512], f16, name=f"qT{p}_{c}") for c in range(NQC)]
                  for p in range(2)]
            kT = [[pp.tile([128, 512], f16, name=f"kT{p}_{c}")
                   if (p == 1 or c < 1) else None for c in range(NQC)]
                  for p in range(2)]

            on_tiles = [None] * NQC  # per-qc [pt] normalized O^T, f16

            def qk_group(dst, wofs, mt, ncu, nw):
                """One q/k projection group: 8 matmuls + DVE copy to SBUF."""
                acc = pa.tile([128, 512], f32, tag="acc", name=f"a{wofs}_{mt}_{ncu}")
                for kt in range(8):
                    w_base = wq_sb if wofs == 0 else wk_sb
                    w_sl = w_base[:, kt * 256 + mt * 128:kt * 256 + (mt + 1) * 128]
                    nc.tensor.matmul(
                        acc[:, 0:nw], w_sl,
                        xsb[ncu][:, kt * 512:kt * 512 + nw],
                        start=(kt == 0), stop=(kt == 7),
                    )
                nc.vector.tensor_copy(out=dst[:, 0:nw], in_=acc[:, 0:nw])

            def v_pair(tt0, ncu):
                """v projection for token tiles tt0, tt0+1 sharing one bank."""
                acc = pb.tile([128, 512], f32, tag="vp", name=f"v{tt0}")
                cols = []
                for i, tt in enumerate((tt0, tt0 + 1)):
                    s0 = _smin(tt)
                    nw = (4 - s0) * 64
                    cols.append((tt, s0, nw))
                    for kt in range(8):
                        nc.tensor.matmul(
                            acc[:, i * 256:i * 256 + nw],
                            xsb[ncu][:, kt * 512 + (tt % 4) * 128:
                                     kt * 512 + (tt % 4 + 1) * 128],
                            wv_sb[:, kt * 256 + s0 * 64:kt * 256 + 256],
                            start=(kt == 0), stop=(kt == 7),
                        )
                for i, (tt, s0, nw) in enumerate(cols):
                    nc.vector.tensor_copy(
                        out=vext[tt][:, s0:4, 0:64],
                        in_=acc[:, i * 256:i * 256 + nw]
                        .rearrange("p (s d) -> p s d", d=64))

            def emit_chunk(ncu):
                # q groups first: they only need wq + x (k needs wkv, which
                # lands on its own queue a bit later at startup)
                with nc.named_scope(f"qkv_c{ncu}"):
                    qk_group(qT[1][ncu], 0, 1, ncu, 512)
                    qk_group(qT[0][ncu], 0, 0, ncu, 512)
                    qk_group(kT[1][ncu], 256, 1, ncu, 512)
                    v_pair(4 * ncu, ncu)
                    v_pair(4 * ncu + 2, ncu)
                    if ncu == 0:
                        qk_group(kT[0][0], 256, 0, 0, 512)

            def emit_proj(qc, last=False):
                with nc.named_scope(f"proj_q{qc}"):
                    for tloc in range(4):
                        tt = qc * 4 + tloc
                        osb = osp.tile([128, 1024], f16, tag="osb", name=f"o{tt}")
                        for ech in range(2):
                            # last proj: ring A is idle by now and has 3 slots
                            # (vs 2) - lets one more start-half matmul run
                            # ahead while the terminal normalize completes
                            pool, tag = (pa, "acc") if last else (pb, "vp")
                            pacc = pool.tile([128, 512], f32, tag=tag,
                                             name=f"pa_{tt}_{ech}")
                            for pt_i in range(2):
                                nc.tensor.matmul(
                                    pacc,
                                    on_tiles[qc][pt_i][:, tloc * 128:(tloc + 1) * 128],
                                    wp_sb[:, pt_i * 1024 + ech * 512:
                                          pt_i * 1024 + (ech + 1) * 512],
                                    start=(pt_i == 0), stop=(pt_i == 1),
                                )
                            nc.vector.tensor_copy(
                                out=osb[:, ech * 512:(ech + 1) * 512], in_=pacc)
                            if last:  # drain the tail on two queues, per half
                                eng = nc.sync if ech == 0 else nc.scalar
                                eng.dma_start(
                                    out=out_d[tt * 128:(tt + 1) * 128,
                                              ech * 512:(ech + 1) * 512],
                                    in_=osb[:, ech * 512:(ech + 1) * 512])
                        if not last:
                            nc.sync.dma_start(
                                out=out_d[tt * 128:(tt + 1) * 128, :], in_=osb)

            nmask = 0

            def emit_attention(qc):
                nonlocal nmask
                on_tiles[qc] = [onp.tile([128, 512], f16, tag="on",
                                         name=f"on_{qc}_{p}") for p in range(2)]
                # last chunk: small pair first so its normalize bounce hides
                # under the big pair's attention; the final slot's normalize
                # uses a PE broadcast (no DMA latency) right before proj.
                pair_order = (0, 1) if qc == NQC - 1 else (1, 0)
                for pair in pair_order:
                    ot_save = [None, None]
                    for s in (2 * pair + 1, 2 * pair):
                        nkt = min(SNKT[s], 4 * qc + 4)
                        r0 = 64 * (s % 2)
                        ot = pc.tile([128, 512], f32, tag="ot", name=f"ot_{qc}_{s}")
                        # terminal pair only: masked diag tiles first, so the
                        # slot ends with clean tiles and the final normalize
                        # chain starts ~1us earlier (global reorder regresses)
                        if qc == NQC - 1 and pair == pair_order[1]:
                            kts = ([k for k in range(nkt) if k >= 4 * qc]
                                   + [k for k in range(nkt) if k < 4 * qc])
                        else:
                            kts = list(range(nkt))
                        with nc.named_scope(f"attn_q{qc}_s{s}"):
                            for ki, kt in enumerate(kts):
                                j0 = (kt - 4 * qc) * 128 if kt >= 4 * qc else 0
                                st = pa.tile([128, 512], f32, tag="acc",
                                             name=f"st_{qc}_{s}_{kt}")
                                nc.tensor.matmul(
                                    st[:, j0:512],
                                    kT[pair][kt // 4][r0:r0 + 64,
                                                      (kt % 4) * 128:(kt % 4 + 1) * 128],
                                    qT[pair][qc][r0:r0 + 64, j0:512],
                                    start=True, stop=True,
                                )
                                p_t = ptp.tile([128, 512], f16, tag="pt",
                                               name=f"pt_{qc}_{s}_{kt}")
                                nc.scalar.activation(
                                    out=p_t[:, j0:512], in_=st[:, j0:512],
                                    func=EXP,
                                    bias=bias_sb[:, s * 16 + kt:s * 16 + kt + 1],
                                    scale=0.125,
                                )
                                if kt >= 4 * qc:
                                    m = kt - 4 * qc
                                    nmask += 1
                                    # split diag-burst masks across gpsimd+DVE.
                                    # Only pair-1 (emitted before the pair's
                                    # DVE norm-muls) may use DVE at qc0: pair-0
                                    # DVE masks would queue behind norm-muls
                                    # stalled on the bounce DMA.
                                    meng = (nc.vector
                                            if (m % 2 == 1 and (qc >= 1 or s >= 2))
                                            else nc.gpsimd)
                                    meng.tensor_mul(
                                        out=p_t[:, j0:512],
                                        in0=p_t[:, j0:512],
                                        in1=mask_sb[:, m * 512 + j0:(m + 1) * 512],
                                    )
                                nc.tensor.matmul(
                                    ot[0:65, j0:512],
                                    vext[kt][:, s, :],
                                    p_t[:, j0:512],
                                    start=(ki == 0), stop=(ki == len(kts) - 1),
                                )
                            ot_save[s % 2] = ot
                    # denominators -> fast reciprocal -> DRAM-bounce partition
                    # broadcast (on the sync queue: its other work, output
                    # tiles, is not latency-critical) -> DVE normalize
                    with nc.named_scope(f"norm_q{qc}_p{pair}"):
                        bcs = bcp.tile([128, 512], f32, tag="bcs",
                                       name=f"b_{qc}_{pair}")
                        for s in (2 * pair + 1, 2 * pair):
                            r0 = 64 * (s % 2)
                            sums = rp.tile([1, 512], f32, tag="sum",
                                           name=f"s_{qc}_{s}")
                            nc.vector.tensor_copy(out=sums,
                                                  in_=ot_save[s % 2][64:65, :])
                            recip = rp.tile([1, 512], f32, tag="rcp",
                                            name=f"r_{qc}_{s}")
                            nc.vector.reciprocal_approx_fast(out=recip, in_=sums)
                            if qc == NQC - 1 and pair == pair_order[1] and s % 2 == 0:
                                # terminal slot: PE matmul broadcast, ~3us
                                # faster than the DMA bounce round trip
                                recip16 = rp.tile([1, 512], f16, tag="r16",
                                                  name=f"r16_{qc}_{s}")
                                nc.vector.tensor_copy(out=recip16, in_=recip)
                                bc_ps = pa.tile([128, 512], f32, tag="acc",
                                                name=f"bps_{qc}_{s}")
                                nc.tensor.matmul(bc_ps[r0:r0 + 64, :],
                                                 ones1[0:1, 0:64], recip16,
                                                 start=True, stop=True)
                                nc.scalar.copy(out=bcs[r0:r0 + 64, :],
                                               in_=bc_ps[r0:r0 + 64, :])
                            else:
                                row = 4 * qc + s
                                nc.sync.dma_start(out=rbounce_d[row:row + 1, :],
                                                  in_=recip)
                                nc.sync.dma_start(
                                    out=bcs[r0:r0 + 64, :],
                                    in_=rbounce_d[row:row + 1, :]
                                    .to_broadcast([64, 512]))
                            nc.vector.tensor_mul(
                                out=on_tiles[qc][pair][r0:r0 + 64, :],
                                in0=ot_save[s % 2][0:64, :],
                                in1=bcs[r0:r0 + 64, :],
                            )

            # last iteration: attention before proj(q2) so the final
            # normalize chains hide under projection matmuls
            for ncu in range(NQC):
                emit_chunk(ncu)
                if 1 <= ncu < NQC - 1:
                    emit_proj(ncu - 1)
                emit_attention(ncu)
            emit_proj(NQC - 2)
            emit_proj(NQC - 1, last=True)

    nc.finalize()
    _NC_CACHE["nc"] = nc
    return nc


def _tile_cols(arr, nblk, blk):
    """[nblk*128, blk] -> [128, nblk*blk] with kt-blocks side by side."""
    n = arr.shape[0] // 128
    assert n == nblk and arr.shape[1] == blk
    return np.ascontiguousarray(
        arr.reshape(nblk, 128, blk).transpose(1, 0, 2).reshape(128, nblk * blk))


def _prep_core_inputs(x, Wq, Aq, Bq, Wk, Ak, Bk, Wv, Av, Bv, Wp):
    """Host-side prep: LoRA fold, transposes, per-core pre-tiled slices."""
    slopes = _slopes()
    wq_m = Wq.astype(np.float64) + LORA_S * (Aq.astype(np.float64) @ Bq.astype(np.float64))
    wk_m = Wk.astype(np.float64) + LORA_S * (Ak.astype(np.float64) @ Bk.astype(np.float64))
    wv_m = Wv.astype(np.float64) + LORA_S * (Av.astype(np.float64) @ Bv.astype(np.float64))

    # mask_m[p, j] = 1 if (m*128 + p) <= j else 0   (j in 0..511)
    p_i = np.arange(128)[:, None]
    j_i = np.arange(512)[None, :]
    masks = np.ascontiguousarray(np.concatenate(
        [((m * 128 + p_i) <= j_i).astype(np.float16) for m in range(4)], axis=1))

    in_maps = []
    for c in range(8):
        b, g = divmod(c, 4)
        heads = [g, 4 + g, 8 + g, 12 + g]
        rows = np.concatenate([np.arange(h * DH, (h + 1) * DH) for h in heads])
        xT = x[b].T.astype(np.float16)          # [E, T]
        x4 = np.concatenate(
            [_tile_cols(np.ascontiguousarray(xT[:, c0 * 512:(c0 + 1) * 512]), 8, 512)
             for c0 in range(NQC)], axis=1)     # [128, 4*4096]
        wqT = _tile_cols(wq_m[rows, :].T.astype(np.float16), 8, 256)
        wkT = _tile_cols(wk_m[rows, :].T.astype(np.float16), 8, 256)
        wvT = _tile_cols(wv_m[rows, :].T.astype(np.float16), 8, 256)
        wpT = _tile_cols(Wp[:, rows].T.astype(np.float16), 2, 1024)
        bias = np.zeros((128, 64), dtype=np.float32)
        for s, h in enumerate(heads):
            for kt in range(16):
                bias[:, s * 16 + kt] = -slopes[h] * (kt * 128 + np.arange(128))
        in_maps.append({
            "x4": np.ascontiguousarray(x4), "wqT": wqT, "wkT": wkT, "wvT": wvT,
            "wpT": wpT, "expbias": bias, "masks": masks,
        })
    return in_maps


def _run(in_maps, trace=False, **kw):
    from concourse.bass_utils import run_bass_kernel_spmd
    nc = _build_nc()
    return run_bass_kernel_spmd(nc, in_maps, core_ids=list(range(8)), trace=trace, **kw)


def kernel(x, Wq, Aq, Bq, Wk, Ak, Bk, Wv, Av, Bv, Wp):
    in_maps = _prep_core_inputs(x, Wq, Aq, Bq, Wk, Ak, Bk, Wv, Av, Bv, Wp)
    res = _run(in_maps)
    out = np.zeros((BATCH, T, E), dtype=np.float32)
    for c in range(8):
        out[c // 4] += res.results[c]["outp"].astype(np.float32)
    return out
